# revision 16
# baseline (speedup 1.0000x reference)
"""ContraNorm Trainium2 kernel: out = 1.2*x - 0.2 * softmax(xn @ xn^T) @ x per batch.

Full input x [8, 2048, 512] f32; batch dim sharded across 8 NeuronCores
(data-parallel, no collectives). Each core runs an identical Bass/Tile program
on its [2048, 512] slice.

Per-core pipeline (bf16 matmul inputs, fp32 PSUM accumulation):
  setup: load x; row sum-of-squares via ACT Square w/ accum_out; rn = 1/sqrt;
         xn = x * rn cast to bf16; PE-transpose xn -> xnT [128, 4, 2048]
         (d on partitions); xe bf16 [128, 16, 520] = x chunks + ones col at 512.
  per 256-row block:
    MM1: S^T[n, m] chunks in PSUM (n on partitions, m on free) -- this makes the
         softmax numerator tiles directly usable as MM2's stationary operand,
         avoiding any attn transpose.
    exp: ACT (no max subtraction needed: sim values are cosines in [-1, 1]).
    MM2: per 128-row half: O = expST.T @ [x | 1] -> PSUM; the ones column
         yields the softmax denominator D at PSUM col 768.
    final: out = 1.2*x + (-0.2/D) * O on DVE; DMA out.
"""

import sys

if "/opt/trn_rl_repo" not in sys.path:
    sys.path.insert(0, "/opt/trn_rl_repo")

from contextlib import ExitStack

import numpy as np

import concourse.bass as bass
import concourse.tile as tile
import concourse.mybir as mybir
from concourse import bacc
from concourse.bass_utils import run_bass_kernel_spmd
from concourse.masks import make_identity

F32 = mybir.dt.float32
BF16 = mybir.dt.bfloat16
AF = mybir.ActivationFunctionType
ALU = mybir.AluOpType

B = 8
P = 128
N = 2048
D = 512
NT = N // P      # 16 row tiles
DS = D // P      # 4 d subtiles
MB = 256         # m superblock (2 row tiles per iter)
ITERS = N // MB  # 8
WCH = 4          # n-chunks per psum wave (2 PSUM banks per wave tile)
WAVES = NT // WCH  # 8

VARIANT = ""  # debug bisect switches, comma-separated
FP8 = mybir.dt.float8e4
USE_FP8 = True  # fp8e4m3 matmul inputs + DoubleRow perf mode (2x PE throughput)


def contranorm_body(ctx: ExitStack, tc: tile.TileContext, out_ap: bass.AP, x_ap: bass.AP):
    nc = tc.nc

    singles = ctx.enter_context(tc.tile_pool(name="singles", bufs=1))
    scratch = ctx.enter_context(tc.tile_pool(name="scratch", bufs=2))
    stats = ctx.enter_context(tc.tile_pool(name="stats", bufs=4))
    xnpool = ctx.enter_context(tc.tile_pool(name="xnpool", bufs=3))

    # persistent tensors
    MDT = FP8 if USE_FP8 else BF16  # matmul input dtype
    xf = singles.tile([P, NT, D], F32)        # x, natural layout (n on partitions)
    xe = singles.tile([P, NT, D + 16], MDT)   # x + ones column at [.., D]
    xnT = singles.tile([P, DS, N], MDT)       # xn transposed (d on partitions)
    # PE transpose path stays bf16 (fp8 transpose needs stride-2 psum writes);
    # the PSUM->SBUF copy casts to MDT.
    ident = singles.tile([P, P], BF16)
    make_identity(nc, ident)
    nc.vector.memset(xe[:, :, D:D + 1], 1.0)

    # PSUM budget (8 banks total, all pools coexist so the scheduler can
    # overlap setup transposes with early main-loop matmuls):
    #   tpsum 2 x 1 bank, psumS 2 x 2 banks, psumO 1 x 2 banks.
    tpsum = ctx.enter_context(tc.tile_pool(name="tpsum", bufs=2, space="PSUM"))
    psumS = ctx.enter_context(tc.tile_pool(name="psumS", bufs=2, space="PSUM"))
    psumO = ctx.enter_context(tc.tile_pool(name="psumO", bufs=1, space="PSUM"))

    # ---------------- setup: norms, xn, transpose ----------------
    variants = set(VARIANT.split(","))
    for i in range(NT):
        nc.sync.dma_start(xf[:, i, :], x_ap[i * P:(i + 1) * P, :])
        # mean/var via DVE bn_stats; ssq/D = var + mean^2
        bst = scratch.tile([P, nc.vector.BN_STATS_DIM], F32, tag="bst")
        nc.vector.bn_stats(bst, xf[:, i, :])
        mv = stats.tile([P, nc.vector.BN_AGGR_DIM], F32, tag="mv")
        nc.vector.bn_aggr(mv, bst)
        vpm = stats.tile([P, 1], F32, tag="vpm")
        nc.vector.tensor_tensor(vpm, mv[:, 0:1], mv[:, 0:1], op=ALU.mult)
        nc.vector.tensor_add(vpm, vpm, mv[:, 1:2])
        # rn = 1/sqrt(vpm * D)  (norms ~22.6 >> eps, the eps clamp is a no-op)
        nrm = stats.tile([P, 1], F32, tag="nrm")
        nc.scalar.activation(nrm, vpm, AF.Sqrt, scale=float(D))
        rn = stats.tile([P, 1], F32, tag="rn")
        nc.vector.reciprocal(rn, nrm)
        # xe chunk: cast x to bf16
        if "actcast" in variants:
            nc.scalar.copy(xe[:, i, 0:D], xf[:, i, :])
        else:
            nc.gpsimd.tensor_copy(xe[:, i, 0:D], xf[:, i, :])
        # xn = x * rn -> bf16 (DVE)
        xn = xnpool.tile([P, D], BF16, tag="xn")
        nc.vector.tensor_scalar_mul(xn, xf[:, i, :], rn)
        # transpose xn into xnT columns [i*P, (i+1)*P)
        if "dvecopy" in variants:
            for dc in range(DS):
                pt = tpsum.tile([P, P], BF16, tag="pt")
                nc.tensor.transpose(pt, xn[:, dc * P:(dc + 1) * P], ident)
                nc.any.tensor_copy(xnT[:, dc, i * P:(i + 1) * P], pt)
        else:
            pt = tpsum.tile([P, DS, P], BF16, tag="pt")
            for dc in range(DS):
                nc.tensor.transpose(pt[:, dc, :], xn[:, dc * P:(dc + 1) * P], ident)
            nc.any.tensor_copy(xnT[:, :, i * P:(i + 1) * P], pt)

    # ---------------- main loop ----------------
    epool = ctx.enter_context(tc.tile_pool(name="epool", bufs=2))
    opool = ctx.enter_context(tc.tile_pool(name="opool", bufs=3))

    for it in range(ITERS):
        mlo = it * MB
        expST = epool.tile([P, NT, MB], MDT, tag="expST")
        for w in range(WAVES):
            ps = psumS.tile([P, WCH, MB], F32, tag="ps")  # 1 bank
            for c in range(WCH):
                j = w * WCH + c  # n-chunk index
                if USE_FP8:
                    for g in range(DS // 2):
                        nc.tensor.matmul(
                            ps[:, c, :],
                            lhsT=xnT[:, 2 * g:2 * g + 2, j * P:(j + 1) * P],
                            rhs=xnT[:, 2 * g:2 * g + 2, mlo:mlo + MB],
                            start=(g == 0),
                            stop=(g == DS // 2 - 1),
                            perf_mode=mybir.MatmulPerfMode.DoubleRow,
                        )
                else:
                    for ds in range(DS):
                        nc.tensor.matmul(
                            ps[:, c, :],
                            lhsT=xnT[:, ds, j * P:(j + 1) * P],
                            rhs=xnT[:, ds, mlo:mlo + MB],
                            start=(ds == 0),
                            stop=(ds == DS - 1),
                        )
            # exp of the whole wave in one ACT instruction
            nc.scalar.activation(expST[:, w * WCH:(w + 1) * WCH, :], ps, AF.Exp)

        for h in range(2):
            i = it * 2 + h  # output row-tile index
            po = psumO.tile([P, 1024], F32, tag="po")  # 2 banks
            if USE_FP8:
                for g in range(NT // 2):
                    lhsT = expST[:, 2 * g:2 * g + 2, h * P:(h + 1) * P]
                    nc.tensor.matmul(po[:, 0:256], lhsT, xe[:, 2 * g:2 * g + 2, 0:256],
                                     start=(g == 0), stop=(g == NT // 2 - 1),
                                     perf_mode=mybir.MatmulPerfMode.DoubleRow)
                    nc.tensor.matmul(po[:, 512:512 + 257], lhsT,
                                     xe[:, 2 * g:2 * g + 2, 256:D + 1],
                                     start=(g == 0), stop=(g == NT // 2 - 1),
                                     perf_mode=mybir.MatmulPerfMode.DoubleRow)
            else:
                for k in range(NT):
                    lhsT = expST[:, k, h * P:(h + 1) * P]
                    nc.tensor.matmul(po[:, 0:256], lhsT, xe[:, k, 0:256],
                                     start=(k == 0), stop=(k == NT - 1))
                    nc.tensor.matmul(po[:, 512:512 + 257], lhsT, xe[:, k, 256:D + 1],
                                     start=(k == 0), stop=(k == NT - 1))
            # s = -0.2 / D  (D at psum col 768)
            sD = stats.tile([P, 1], F32, tag="sD")
            nc.vector.tensor_scalar_mul(sD, po[:, 768:769], -5.0)
            rD = stats.tile([P, 1], F32, tag="rD")
            nc.vector.reciprocal(rD, sD)
            # tmp = O * s ; O cols are [0:256] and [512:768]
            tmp = opool.tile([P, 2, 256], F32, tag="tmp")
            po3 = po.rearrange("p (b c) -> p b c", b=2, c=512)[:, :, 0:256]
            nc.vector.tensor_scalar_mul(tmp, po3, rD)
            # out = x * 1.2 + tmp
            ob = opool.tile([P, D], F32, tag="ob")
            nc.vector.scalar_tensor_tensor(
                ob, xf[:, i, :], 1.2, tmp.rearrange("p b c -> p (b c)"),
                op0=ALU.mult, op1=ALU.add)
            nc.sync.dma_start(out_ap[i * P:(i + 1) * P, :], ob)


def build_nc(repeats: int = 1):
    """Build + compile the per-core Bass program. `repeats` re-emits the body
    (sharing pools/SBUF) for steady-state timing measurements."""
    nc = bacc.Bacc("TRN2", target_bir_lowering=False, debug=False, enable_asserts=False)
    x = nc.dram_tensor("x", [N, D], F32, kind="ExternalInput").ap()
    out = nc.dram_tensor("out", [N, D], F32, kind="ExternalOutput").ap()
    with tile.TileContext(nc) as tc:
        for _ in range(repeats):
            with ExitStack() as ctx:
                contranorm_body(ctx, tc, out, x)
    nc.compile()
    return nc


_nc_cache = {}


def kernel(x: np.ndarray) -> np.ndarray:
    assert x.shape == (B, N, D), x.shape
    x = np.ascontiguousarray(x, dtype=np.float32)
    if "nc" not in _nc_cache:
        _nc_cache["nc"] = build_nc()
    nc = _nc_cache["nc"]
    in_maps = [{"x": x[i]} for i in range(B)]
    res = run_bass_kernel_spmd(nc, in_maps, core_ids=list(range(B)))
    return np.stack([r["out"] for r in res.results], axis=0)


# revision 18
# speedup vs baseline: 2.8496x; 2.8496x over previous
"""ContraNorm Trainium2 kernel: out = 1.2*x - 0.2 * softmax(xn @ xn^T) @ x per batch.

Full input x [8, 2048, 512] f32; batch dim sharded across 8 NeuronCores
(data-parallel, no collectives). Each core runs an identical Bass/Tile program
on its [2048, 512] slice.

Per-core pipeline (bf16 matmul inputs, fp32 PSUM accumulation):
  setup: load x; row sum-of-squares via ACT Square w/ accum_out; rn = 1/sqrt;
         xn = x * rn cast to bf16; PE-transpose xn -> xnT [128, 4, 2048]
         (d on partitions); xe bf16 [128, 16, 520] = x chunks + ones col at 512.
  per 256-row block:
    MM1: S^T[n, m] chunks in PSUM (n on partitions, m on free) -- this makes the
         softmax numerator tiles directly usable as MM2's stationary operand,
         avoiding any attn transpose.
    exp: ACT (no max subtraction needed: sim values are cosines in [-1, 1]).
    MM2: per 128-row half: O = expST.T @ [x | 1] -> PSUM; the ones column
         yields the softmax denominator D at PSUM col 768.
    final: out = 1.2*x + (-0.2/D) * O on DVE; DMA out.
"""

import sys

if "/opt/trn_rl_repo" not in sys.path:
    sys.path.insert(0, "/opt/trn_rl_repo")

from contextlib import ExitStack

import numpy as np

import concourse.bass as bass
import concourse.tile as tile
import concourse.mybir as mybir
from concourse import bacc
from concourse.bass_utils import run_bass_kernel_spmd

F32 = mybir.dt.float32
BF16 = mybir.dt.bfloat16
AF = mybir.ActivationFunctionType
ALU = mybir.AluOpType

B = 8
P = 128
N = 2048
D = 512
NT = N // P      # 16 row tiles
DS = D // P      # 4 d subtiles
MB = 256         # m superblock (2 row tiles per iter)
ITERS = N // MB  # 8
WCH = 4          # n-chunks per psum wave (2 PSUM banks per wave tile)
WAVES = NT // WCH  # 8

VARIANT = ""  # debug bisect switches, comma-separated
FP8 = mybir.dt.float8e4
USE_FP8 = True  # fp8e4m3 matmul inputs + DoubleRow perf mode (2x PE throughput)


def contranorm_body(ctx: ExitStack, tc: tile.TileContext, out_ap: bass.AP, x_ap: bass.AP):
    nc = tc.nc

    singles = ctx.enter_context(tc.tile_pool(name="singles", bufs=1))
    scratch = ctx.enter_context(tc.tile_pool(name="scratch", bufs=3))
    stats = ctx.enter_context(tc.tile_pool(name="stats", bufs=8))
    xnpool = ctx.enter_context(tc.tile_pool(name="xnpool", bufs=4))

    # persistent tensors
    MDT = FP8 if USE_FP8 else BF16  # matmul input dtype
    xf = singles.tile([P, NT, D], F32)        # x, natural layout (n on partitions)
    xe = singles.tile([P, NT, D + 16], MDT)   # x + ones column at [.., D]
    xnT = singles.tile([P, DS, N], MDT)       # xn transposed (d on partitions)
    xnTb = singles.tile([P, DS, N], BF16)     # bf16 staging for the DMA transpose
    nc.vector.memset(xe[:, :, D:D + 1], 1.0)

    # PSUM budget (8 banks total): psumS 3 x 2 banks, psumO 1 x 2 banks.
    psumS = ctx.enter_context(tc.tile_pool(name="psumS", bufs=3, space="PSUM"))
    psumO = ctx.enter_context(tc.tile_pool(name="psumO", bufs=1, space="PSUM"))

    # ---------------- setup: norms, xn, transpose ----------------
    variants = set(VARIANT.split(","))
    for i in range(NT):
        nc.sync.dma_start(xf[:, i, :], x_ap[i * P:(i + 1) * P, :])
        # mean/var via DVE bn_stats; ssq/D = var + mean^2
        bst = scratch.tile([P, nc.vector.BN_STATS_DIM], F32, tag="bst")
        nc.vector.bn_stats(bst, xf[:, i, :])
        mv = stats.tile([P, nc.vector.BN_AGGR_DIM], F32, tag="mv")
        nc.vector.bn_aggr(mv, bst)
        vpm = stats.tile([P, 1], F32, tag="vpm")
        nc.vector.tensor_tensor(vpm, mv[:, 0:1], mv[:, 0:1], op=ALU.mult)
        nc.vector.tensor_add(vpm, vpm, mv[:, 1:2])
        # rn = 1/sqrt(vpm * D)  (norms ~22.6 >> eps, the eps clamp is a no-op)
        nrm = stats.tile([P, 1], F32, tag="nrm")
        nc.scalar.activation(nrm, vpm, AF.Sqrt, scale=float(D))
        rn = stats.tile([P, 1], F32, tag="rn")
        nc.vector.reciprocal(rn, nrm)
        # xe chunk: cast x to bf16
        if "actcast" in variants:
            nc.scalar.copy(xe[:, i, 0:D], xf[:, i, :])
        else:
            nc.gpsimd.tensor_copy(xe[:, i, 0:D], xf[:, i, :])
        # xn = x * rn -> bf16 (DVE)
        xn = xnpool.tile([P, D], BF16, tag="xn")
        nc.vector.tensor_scalar_mul(xn, xf[:, i, :], rn)
        # transpose xn into xnT columns [i*P, (i+1)*P): DMA-transpose the bf16
        # tile, then GPSIMD casts bf16 -> fp8 (keeps ACT/DVE off this path)
        nc.sync.dma_start_transpose(xnTb[:, :, i * P:(i + 1) * P], xn)
        nc.gpsimd.tensor_copy(xnT[:, :, i * P:(i + 1) * P],
                               xnTb[:, :, i * P:(i + 1) * P])

    # ---------------- main loop ----------------
    epool = ctx.enter_context(tc.tile_pool(name="epool", bufs=3))
    opool = ctx.enter_context(tc.tile_pool(name="opool", bufs=3))

    for it in range(ITERS):
        mlo = it * MB
        expST = epool.tile([P, NT, MB], MDT, tag="expST")
        for w in range(WAVES):
            ps = psumS.tile([P, WCH, MB], F32, tag="ps")  # 1 bank
            for c in range(WCH):
                j = w * WCH + c  # n-chunk index
                if USE_FP8:
                    for g in range(DS // 2):
                        nc.tensor.matmul(
                            ps[:, c, :],
                            lhsT=xnT[:, 2 * g:2 * g + 2, j * P:(j + 1) * P],
                            rhs=xnT[:, 2 * g:2 * g + 2, mlo:mlo + MB],
                            start=(g == 0),
                            stop=(g == DS // 2 - 1),
                            perf_mode=mybir.MatmulPerfMode.DoubleRow,
                        )
                else:
                    for ds in range(DS):
                        nc.tensor.matmul(
                            ps[:, c, :],
                            lhsT=xnT[:, ds, j * P:(j + 1) * P],
                            rhs=xnT[:, ds, mlo:mlo + MB],
                            start=(ds == 0),
                            stop=(ds == DS - 1),
                        )
            # exp of the whole wave in one ACT instruction
            nc.scalar.activation(expST[:, w * WCH:(w + 1) * WCH, :], ps, AF.Exp)

        for h in range(2):
            i = it * 2 + h  # output row-tile index
            po = psumO.tile([P, 1024], F32, tag="po")  # 2 banks
            if USE_FP8:
                for g in range(NT // 2):
                    lhsT = expST[:, 2 * g:2 * g + 2, h * P:(h + 1) * P]
                    nc.tensor.matmul(po[:, 0:256], lhsT, xe[:, 2 * g:2 * g + 2, 0:256],
                                     start=(g == 0), stop=(g == NT // 2 - 1),
                                     perf_mode=mybir.MatmulPerfMode.DoubleRow)
                    nc.tensor.matmul(po[:, 512:512 + 257], lhsT,
                                     xe[:, 2 * g:2 * g + 2, 256:D + 1],
                                     start=(g == 0), stop=(g == NT // 2 - 1),
                                     perf_mode=mybir.MatmulPerfMode.DoubleRow)
            else:
                for k in range(NT):
                    lhsT = expST[:, k, h * P:(h + 1) * P]
                    nc.tensor.matmul(po[:, 0:256], lhsT, xe[:, k, 0:256],
                                     start=(k == 0), stop=(k == NT - 1))
                    nc.tensor.matmul(po[:, 512:512 + 257], lhsT, xe[:, k, 256:D + 1],
                                     start=(k == 0), stop=(k == NT - 1))
            # s = -0.2 / D  (D at psum col 768)
            sD = stats.tile([P, 1], F32, tag="sD")
            nc.vector.tensor_scalar_mul(sD, po[:, 768:769], -5.0)
            rD = stats.tile([P, 1], F32, tag="rD")
            nc.vector.reciprocal(rD, sD)
            # tmp = O * s ; O cols are [0:256] and [512:768]
            tmp = opool.tile([P, 2, 256], F32, tag="tmp")
            po3 = po.rearrange("p (b c) -> p b c", b=2, c=512)[:, :, 0:256]
            nc.vector.tensor_scalar_mul(tmp, po3, rD)
            # out = x * 1.2 + tmp
            ob = opool.tile([P, D], F32, tag="ob")
            nc.vector.scalar_tensor_tensor(
                ob, xf[:, i, :], 1.2, tmp.rearrange("p b c -> p (b c)"),
                op0=ALU.mult, op1=ALU.add)
            nc.sync.dma_start(out_ap[i * P:(i + 1) * P, :], ob)


def build_nc(repeats: int = 1):
    """Build + compile the per-core Bass program. `repeats` re-emits the body
    (sharing pools/SBUF) for steady-state timing measurements."""
    nc = bacc.Bacc("TRN2", target_bir_lowering=False, debug=False, enable_asserts=False)
    x = nc.dram_tensor("x", [N, D], F32, kind="ExternalInput").ap()
    out = nc.dram_tensor("out", [N, D], F32, kind="ExternalOutput").ap()
    with tile.TileContext(nc) as tc:
        for _ in range(repeats):
            with ExitStack() as ctx:
                contranorm_body(ctx, tc, out, x)
    nc.compile()
    return nc


_nc_cache = {}


def kernel(x: np.ndarray) -> np.ndarray:
    assert x.shape == (B, N, D), x.shape
    x = np.ascontiguousarray(x, dtype=np.float32)
    if "nc" not in _nc_cache:
        _nc_cache["nc"] = build_nc()
    nc = _nc_cache["nc"]
    in_maps = [{"x": x[i]} for i in range(B)]
    res = run_bass_kernel_spmd(nc, in_maps, core_ids=list(range(B)))
    return np.stack([r["out"] for r in res.results], axis=0)
